# revision 14
# baseline (speedup 1.0000x reference)
"""GNN message-passing (CG-CNN layer) Trainium2 kernel.

out = feature + segment_sum(sigmoid(msg@Wf+bf) * softplus(msg@Ws+bs), dst)
where msg = [feature[src], feature[dst], dist].

Strategy (8 NeuronCores, SPMD, no collectives):
- Shard nodes by dst-range: core c owns nodes [c*6254, (c+1)*6254). Each core
  receives exactly the edges whose dst is in its range, grouped into windows
  of 118 dst-nodes.
- Per-node precompute (on device): P[n] = feature[n] @ [-Wf_src | Ws_src] and
  Q[n] = feature[n] @ [-Wf_dst | Ws_dst] + [-bf | bs]  (fp16 tables in DRAM).
  The f-half signs are flipped so the psum holds [-zf | zs] and a single
  full-width exp() serves both gates.
- Per edge group of 128: z = MM_A(L_g, R_w) + MM_B(I, P[src]-gather) where
  L_g = [onehot(dst_local) ; dist^T] (host-built fp16) and R_w = [Q_win ; Wd].
- gated = ln(1+e^{zs}) * 1/(1+e^{-zf}) using the exp/ln ACT table set only.
- scatter: m_sum_win += onehot^T @ gated via PE matmul accumulation in PSUM
  (race-free), then out_win = m_sum + feature[win] written densely.
"""

import sys

sys.path.insert(0, "/opt/trn_rl_repo")

import numpy as np

F16 = np.float16

# ---------------------------------------------------------------- problem dims
N_NODES = 50000
N_EDGES = 800000
F = 64
D = 10
NC = 8

WIN = 118          # dst-nodes per window (K budget: 118 + 10 = 128)
WPC = 53           # windows per core
HALF = 32768       # int16 gather index budget
BATCH = 8          # edge groups per psum batch
SLAB = 1024        # node-table build slab


def _cdiv(a, b):
    return (a + b - 1) // b


def _cdiv_arr(a, b):
    return -(-a // b)


# ============================================================ host preprocessing
def preprocess(feature, dist, src, dst, Wf, bf, Ws, bs,
               n_nodes=N_NODES, nc_cores=NC, win=WIN, wpc=WPC, half=HALF):
    """Pure layout/indexing prep on host. Returns (per_core_inputs, meta)."""
    nodes_pc = win * wpc
    assert nc_cores * nodes_pc >= n_nodes

    feature = np.asarray(feature, np.float32)
    dist = np.asarray(dist, np.float32)
    src = np.asarray(src).astype(np.int64)
    dst = np.asarray(dst).astype(np.int64)
    Wf = np.asarray(Wf, np.float32)
    bf = np.asarray(bf, np.float32)
    Ws = np.asarray(Ws, np.float32)
    bs = np.asarray(bs, np.float32)

    n_tab = _cdiv(n_nodes, SLAB) * SLAB          # P table rows (slab-aligned)
    qrows = _cdiv(nodes_pc, 128) * 128           # Q table rows per core

    core = dst // nodes_pc
    loc = dst - core * nodes_pc
    w = loc // win
    n_in_w = loc - w * win
    hi = (src >= half).astype(np.int64)

    key = (core * wpc + w) * 2 + hi
    order = np.argsort(key, kind="stable")
    counts = np.bincount(key, minlength=nc_cores * wpc * 2).reshape(nc_cores, wpc, 2)

    g_lo = _cdiv_arr(counts[:, :, 0].max(axis=0), 128)
    g_hi = _cdiv_arr(counts[:, :, 1].max(axis=0), 128)
    empty = (g_lo + g_hi) == 0
    g_lo[empty] = 1                               # every window has >=1 group
    s_w = g_lo + g_hi
    base_g = np.concatenate([[0], np.cumsum(s_w)])
    totg = int(base_g[-1])
    totslots = totg * 128
    smax = int(s_w.max())

    # weight tables (f-half negated so psum = [-zf | zs])
    wsrc = np.concatenate([-Wf[0:F], Ws[0:F]], axis=1).astype(F16)          # [64,128]
    wdst = np.concatenate([-Wf[F:2 * F], Ws[F:2 * F]], axis=1)
    bcat = np.concatenate([-bf, bs])[None, :]
    wq = np.concatenate([wdst, bcat], axis=0).astype(F16)                   # [65,128]
    wd = np.concatenate([-Wf[2 * F:], Ws[2 * F:]], axis=1).astype(F16)      # [10,128]
    ident = np.eye(128, dtype=F16)
    iota = np.tile(np.arange(128, dtype=F16)[None, :], (128, smax))         # [128,smax*128]

    # feature tables padded to 128 cols for xbar DMA-transpose
    feat16 = np.zeros((n_tab, 128), F16)
    feat16[:n_nodes, :F] = feature.astype(F16)

    per_core = []
    core_s, w_s, hi_s = core[order], w[order], hi[order]
    src_s, niw_s = src[order], n_in_w[order]
    dist_s = dist[order]

    for c in range(nc_cores):
        sidx = np.zeros(totslots, np.int16)
        Lhost = np.zeros((128, totslots), F16)
        dstloc = np.full(totslots, -5.0, F16)   # flat (g*128+p)

        sel = core_s == c
        cw, chi, csrc, cniw = w_s[sel], hi_s[sel], src_s[sel], niw_s[sel]
        cdist = dist_s[sel]
        ckey = cw * 2 + chi
        cnt = counts[c].reshape(-1)
        off = np.concatenate([[0], np.cumsum(cnt)])
        pos = np.arange(len(ckey)) - off[ckey]
        gcol = base_g[cw] + np.where(chi == 1, g_lo[cw], 0) + pos // 128
        p = pos % 128
        slot = gcol * 128 + p

        sidx[slot] = (csrc - np.where(chi == 1, half, 0)).astype(np.int16)
        dstloc[slot] = cniw.astype(F16)
        Lhost[cniw, slot] = 1.0
        Lhost[win + np.arange(D)[:, None], slot[None, :]] = cdist.T.astype(F16)

        frange = np.zeros((nodes_pc, F), np.float32)
        lo_n, hi_n = c * nodes_pc, min((c + 1) * nodes_pc, n_nodes)
        if hi_n > lo_n:
            frange[: hi_n - lo_n] = feature[lo_n:hi_n]
        featloc = np.zeros((qrows, 128), F16)
        featloc[:, F] = 1.0                      # bias-ones row after transpose
        hi_q = min(lo_n + qrows, n_nodes)
        if hi_q > lo_n:
            featloc[: hi_q - lo_n, :F] = feature[lo_n:hi_q].astype(F16)

        per_core.append({
            "feat16": feat16,
            "featloc": featloc,
            "frange": frange,
            "sidx": np.tile(sidx.reshape(totslots // 16, 16).T, (8, 1)).copy(),
            "Lmat": Lhost,
            "dstloc": dstloc.reshape(totg, 128).T.copy(),
            "wsrc": wsrc, "wq": wq, "wd": wd,
            "ident": ident, "iota": iota,
        })

    meta = {
        "g_lo": g_lo.tolist(), "g_hi": g_hi.tolist(),
        "base_g": base_g.tolist(), "totg": totg, "smax": smax,
        "n_tab": n_tab, "qrows": qrows, "win": win, "wpc": wpc,
        "nodes_pc": nodes_pc, "half": half,
    }
    return per_core, meta


# ============================================================== program builder
def build_program(meta, nc_cores=NC, phases=('pq', 'edge'), repeat=1):
    import concourse.tile as tile
    import concourse.mybir as mybir
    from concourse import bacc
    from concourse.bass import ts

    dt = mybir.dt
    AF = mybir.ActivationFunctionType
    ALU = mybir.AluOpType

    g_lo, g_hi = meta["g_lo"], meta["g_hi"]
    base_g = meta["base_g"]
    totg, smax = meta["totg"], meta["smax"]
    n_tab, qrows = meta["n_tab"], meta["qrows"]
    win, wpc, nodes_pc, half = meta["win"], meta["wpc"], meta["nodes_pc"], meta["half"]
    totslots = totg * 128

    import concourse.mybir as _mb
    import bass_rust as _br

    class _Bacc(bacc.Bacc):
        # Pin every activation to the one set holding Copy+Exp+Ln so the
        # table pass emits a single load instead of thrashing (2.7us/load).
        def insert_act_table_loads(self):
            from concourse.hw_specs import get_activation_tables
            has_act = any(isinstance(i, _mb.InstActivation)
                          for b in self.main_func.blocks for i in b.instructions)
            if not has_act:
                return
            tables = list(get_activation_tables(self.m.arch).items())
            keep = "natural_log_exp_and_others"
            filtered = [(n, (f if n == keep else set())) for n, f in tables]
            _br.insert_act_table_loads(self, filtered)

    nc = _Bacc("TRN2", target_bir_lowering=False, debug=False,
               num_devices=nc_cores, num_swdge_queues=4)

    f16, f32, i16 = dt.float16, dt.float32, dt.int16

    feat16 = nc.dram_tensor("feat16", [n_tab, 128], f16, kind="ExternalInput").ap()
    featloc = nc.dram_tensor("featloc", [qrows, 128], f16, kind="ExternalInput").ap()
    frange = nc.dram_tensor("frange", [nodes_pc, F], f32, kind="ExternalInput").ap()
    sidx_d = nc.dram_tensor("sidx", [128, totslots // 16], i16, kind="ExternalInput").ap()
    L_d = nc.dram_tensor("Lmat", [128, totslots], f16, kind="ExternalInput").ap()
    dstloc_d = nc.dram_tensor("dstloc", [128, totg], f16, kind="ExternalInput").ap()
    wsrc_d = nc.dram_tensor("wsrc", [F, 128], f16, kind="ExternalInput").ap()
    wq_d = nc.dram_tensor("wq", [F + 1, 128], f16, kind="ExternalInput").ap()
    wd_d = nc.dram_tensor("wd", [D, 128], f16, kind="ExternalInput").ap()
    ident_d = nc.dram_tensor("ident", [128, 128], f16, kind="ExternalInput").ap()
    iota_d = nc.dram_tensor("iota", [128, smax * 128], f16, kind="ExternalInput").ap()
    out_d = nc.dram_tensor("out", [nodes_pc, F], f32, kind="ExternalOutput").ap()

    P_d = nc.dram_tensor("P_tab", [n_tab, 128], f16).ap()
    Q_d = nc.dram_tensor("Q_tab", [qrows, 128], f16).ap()

    with tile.TileContext(nc) as tc:
        from contextlib import ExitStack
        with ExitStack() as ctx:
            if repeat > 1:
                ctx.enter_context(tc.For_i(0, repeat, 1))
            consts = ctx.enter_context(tc.tile_pool(name="consts", bufs=1))
            wsrc_t = consts.tile([F, 128], f16)
            nc.sync.dma_start(wsrc_t[:], wsrc_d[:])
            wq_t = consts.tile([F + 1, 128], f16)
            nc.sync.dma_start(wq_t[:], wq_d[:])
            ident_t = consts.tile([128, 128], f16)
            nc.sync.dma_start(ident_t[:], ident_d[:])
            iota_t = consts.tile([128, smax * 128], f16)
            nc.sync.dma_start(iota_t[:], iota_d[:])

            # ---------------- phase P/Q: node tables -------------------------
            with tc.tile_pool(name="pphase", bufs=3) as pp, \
                 tc.tile_pool(name="ppsum", bufs=2, space="PSUM") as pps:
                for a in range(0, n_tab, SLAB):
                    featT = pp.tile([128, SLAB], f16, tag="featT")
                    nc.sync.dma_start(featT[:], feat16[a:a + SLAB, :], transpose=True)
                    psum = pps.tile([128, SLAB], f32, tag="pp")
                    for j in range(SLAB // 128):
                        nc.tensor.matmul(psum[:, ts(j, 128)],
                                         lhsT=featT[0:F, ts(j, 128)],
                                         rhs=wsrc_t[:], start=True, stop=True)
                    pout = pp.tile([128, SLAB], f16, tag="pout")
                    nc.scalar.activation(pout[:], psum[:], AF.Copy)
                    nc.sync.dma_start(
                        P_d[a:a + SLAB, :].rearrange("(j p) f -> p j f", p=128),
                        pout[:].rearrange("p (j f) -> p j f", f=128))

                for a in range(0, qrows, SLAB):
                    sz = min(SLAB, qrows - a)
                    featq = pp.tile([128, SLAB], f16, tag="featT")
                    nc.sync.dma_start(featq[:, 0:sz], featloc[a:a + sz, :],
                                      transpose=True)
                    psum = pps.tile([128, SLAB], f32, tag="pp")
                    for j in range(sz // 128):
                        nc.tensor.matmul(psum[:, ts(j, 128)],
                                         lhsT=featq[0:F + 1, ts(j, 128)],
                                         rhs=wq_t[:], start=True, stop=True)
                    qout = pp.tile([128, SLAB], f16, tag="pout")
                    nc.scalar.activation(qout[:, 0:sz], psum[:, 0:sz], AF.Copy)
                    nc.sync.dma_start(
                        Q_d[a:a + sz, :].rearrange("(j p) f -> p j f", p=128),
                        qout[:, 0:sz].rearrange("p (j f) -> p j f", f=128))

            # ---------------- phase E: edges ---------------------------------
            if 'edge' not in phases:
                with tc.tile_pool(name='dummy', bufs=1) as dp:
                    zt = dp.tile([128, F], dt.float32)
                    nc.gpsimd.memset(zt[:], 0.0)
                    for w in range(wpc):
                        nc.sync.dma_start(out_d[w * win:(w + 1) * win, :], zt[0:win, :])
                return_early = True
            else:
                return_early = False
            if not return_early:
             with tc.tile_pool(name="ewin", bufs=4) as ew, \
                 tc.tile_pool(name="ebatch", bufs=6) as eb, \
                 tc.tile_pool(name="zpsum", bufs=3, space="PSUM") as zps, \
                 tc.tile_pool(name="mpsum", bufs=2, space="PSUM") as mps, \
                 tc.tile_pool(name="eout", bufs=3) as eo:
                _gq = [0]
                for w in range(wpc):
                    glo, ghi = g_lo[w], g_hi[w]
                    S = glo + ghi
                    g0 = base_g[w]

                    gsrc = ew.tile([128, S * 128], f16, tag="gsrc")
                    sidx_t = ew.tile([128, S * 8], i16, tag="sidx")
                    nc.sync.dma_start(sidx_t[:], sidx_d[:, g0 * 8:(g0 + S) * 8])
                    if 'nogather' in phases:
                        nc.gpsimd.memset(gsrc[:], 0.25)
                    if 'nogather' not in phases:
                        GMAX = 8   # ring holds 1024 descriptors per call
                        spans = [(g, min(g + GMAX, glo), P_d[0:min(half, n_tab), :])
                                 for g in range(0, glo, GMAX)]
                        spans += [(g, min(g + GMAX, S), P_d[half:n_tab, :])
                                  for g in range(glo, S, GMAX)]
                        for ga, gb, src_ap in spans:
                            ng = gb - ga
                            nc.gpsimd.dma_gather(
                                out_ap=gsrc[:, ga * 128:gb * 128]
                                .rearrange("p (c e) -> p c e", e=128),
                                in_ap=src_ap,
                                idxs_ap=sidx_t[:, ga * 8:gb * 8],
                                num_idxs=ng * 128, num_idxs_reg=ng * 128,
                                elem_size=128, queue_num=_gq[0] % 4)
                            _gq[0] += 1

                    L_t = ew.tile([128, S * 128], f16, tag="L")
                    nc.sync.dma_start(L_t[:], L_d[:, g0 * 128:(g0 + S) * 128])
                    dl_t = ew.tile([128, S], f16, tag="dl")
                    nc.sync.dma_start(dl_t[:], dstloc_d[:, g0:g0 + S])
                    R_t = ew.tile([128, 128], f16, tag="R")
                    nc.sync.dma_start(R_t[0:win, :],
                                      Q_d[w * win:(w + 1) * win, :])
                    nc.sync.dma_start(R_t[win:128, :], wd_d[:])

                    oh_t = ew.tile([128, S * 128], f16, tag="oh")
                    nc.vector.tensor_tensor(
                        out=oh_t[:].rearrange("p (s e) -> p s e", e=128),
                        in0=dl_t[:, :, None].to_broadcast([128, S, 128]),
                        in1=iota_t[:, 0:S * 128].rearrange("p (s e) -> p s e", e=128),
                        op=ALU.is_equal)

                    msum = mps.tile([win, F], f32, tag="msum")

                    for b0 in range(0, S, BATCH):
                        nb = min(BATCH, S - b0)
                        zp = zps.tile([128, BATCH * 128], f32, tag="zp")
                        for j in range(nb):
                            g = b0 + j
                            nc.tensor.matmul(zp[:, ts(j, 128)],
                                             lhsT=L_t[:, ts(g, 128)], rhs=R_t[:],
                                             start=True, stop=False)
                            nc.tensor.matmul(zp[:, ts(j, 128)], lhsT=ident_t[:],
                                             rhs=gsrc[:, ts(g, 128)],
                                             start=False, stop=True)
                        ez = eb.tile([128, BATCH * 128], f16, tag="ez")
                        nc.scalar.activation(ez[:, 0:nb * 128], zp[:, 0:nb * 128],
                                             AF.Exp)
                        # sp = ln(1 + e^{zs});  d = 1 + e^{-zf};  gated = sp/d
                        ezv = ez[:, 0:nb * 128].rearrange("p (j e) -> p j e", e=128)
                        sp_t = eb.tile([128, BATCH * F], f16, tag="sp")
                        spv = sp_t[:, 0:nb * F].rearrange("p (j e) -> p j e", e=F)
                        nc.scalar.activation(spv, ezv[:, :, F:128], AF.Ln, bias=1.0)
                        d_t = eb.tile([128, BATCH * F], f32, tag="d")
                        dv = d_t[:, 0:nb * F].rearrange("p (j e) -> p j e", e=F)
                        nc.vector.tensor_scalar(out=dv, in0=ezv[:, :, 0:F],
                                                scalar1=1.0, scalar2=None,
                                                op0=ALU.add)
                        r_t = eb.tile([128, BATCH * F], f32, tag="r")
                        nc.vector.reciprocal_approx_fast(r_t[:, 0:nb * F],
                                                         d_t[:, 0:nb * F])
                        gat = eb.tile([128, BATCH * F], f16, tag="gat")
                        nc.vector.tensor_tensor(out=gat[:, 0:nb * F],
                                                in0=sp_t[:, 0:nb * F],
                                                in1=r_t[:, 0:nb * F], op=ALU.mult)
                        gatv = gat[:, 0:nb * F].rearrange("p (j e) -> p j e", e=F)
                        for j in range(nb):
                            g = b0 + j
                            nc.tensor.matmul(
                                msum[:], lhsT=oh_t[:, g * 128:g * 128 + win],
                                rhs=gatv[:, j, :],
                                start=(g == 0), stop=(g == S - 1))

                    fr_t = eo.tile([win, F], f32, tag="fr")
                    nc.sync.dma_start(fr_t[:], frange[w * win:(w + 1) * win, :])
                    o_t = eo.tile([win, F], f32, tag="o")
                    nc.vector.tensor_tensor(out=o_t[:], in0=msum[:], in1=fr_t[:],
                                            op=ALU.add)
                    nc.sync.dma_start(out_d[w * win:(w + 1) * win, :], o_t[:])

    nc.compile()
    return nc


# ===================================================================== kernel()
_CACHE = {}


def kernel(**inputs):
    per_core, meta = preprocess(
        inputs["feature"], inputs["dist"], inputs["src"], inputs["dst"],
        inputs["Wf"], inputs["bf"], inputs["Ws"], inputs["bs"])

    key = (meta["totg"], tuple(meta["g_lo"]), tuple(meta["g_hi"]))
    if key not in _CACHE:
        _CACHE.clear()
        _CACHE[key] = build_program(meta)
    nc = _CACHE[key]

    from concourse.bass_utils import run_bass_kernel_spmd
    res = run_bass_kernel_spmd(nc, per_core, list(range(NC)))

    outs = [res.results[c]["out"] for c in range(NC)]
    full = np.concatenate(outs, axis=0)[:N_NODES]
    return np.asarray(full, np.float32)


# revision 15
# speedup vs baseline: 1.0370x; 1.0370x over previous
"""GNN message-passing (CG-CNN layer) Trainium2 kernel.

out = feature + segment_sum(sigmoid(msg@Wf+bf) * softplus(msg@Ws+bs), dst)
where msg = [feature[src], feature[dst], dist].

Strategy (8 NeuronCores, SPMD, no collectives):
- Shard nodes by dst-range: core c owns nodes [c*6254, (c+1)*6254). Each core
  receives exactly the edges whose dst is in its range, grouped into windows
  of 118 dst-nodes.
- Per-node precompute (on device): P[n] = feature[n] @ [-Wf_src | Ws_src] and
  Q[n] = feature[n] @ [-Wf_dst | Ws_dst] + [-bf | bs]  (fp16 tables in DRAM).
  The f-half signs are flipped so the psum holds [-zf | zs] and a single
  full-width exp() serves both gates.
- Per edge group of 128: z = MM_A(L_g, R_w) + MM_B(I, P[src]-gather) where
  L_g = [onehot(dst_local) ; dist^T] (host-built fp16) and R_w = [Q_win ; Wd].
- gated = ln(1+e^{zs}) * 1/(1+e^{-zf}) using the exp/ln ACT table set only.
- scatter: m_sum_win += onehot^T @ gated via PE matmul accumulation in PSUM
  (race-free), then out_win = m_sum + feature[win] written densely.
"""

import sys

sys.path.insert(0, "/opt/trn_rl_repo")

import numpy as np

F16 = np.float16

# ---------------------------------------------------------------- problem dims
N_NODES = 50000
N_EDGES = 800000
F = 64
D = 10
NC = 8

WIN = 118          # dst-nodes per window (K budget: 118 + 10 = 128)
WPC = 53           # windows per core
HALF = 32768       # int16 gather index budget
BATCH = 8          # edge groups per psum batch
SLAB = 1024        # node-table build slab


def _cdiv(a, b):
    return (a + b - 1) // b


def _cdiv_arr(a, b):
    return -(-a // b)


# ============================================================ host preprocessing
def preprocess(feature, dist, src, dst, Wf, bf, Ws, bs,
               n_nodes=N_NODES, nc_cores=NC, win=WIN, wpc=WPC, half=HALF):
    """Pure layout/indexing prep on host. Returns (per_core_inputs, meta)."""
    nodes_pc = win * wpc
    assert nc_cores * nodes_pc >= n_nodes

    feature = np.asarray(feature, np.float32)
    dist = np.asarray(dist, np.float32)
    src = np.asarray(src).astype(np.int64)
    dst = np.asarray(dst).astype(np.int64)
    Wf = np.asarray(Wf, np.float32)
    bf = np.asarray(bf, np.float32)
    Ws = np.asarray(Ws, np.float32)
    bs = np.asarray(bs, np.float32)

    n_tab = _cdiv(n_nodes, SLAB) * SLAB          # P table rows (slab-aligned)
    qrows = _cdiv(nodes_pc, 128) * 128           # Q table rows per core

    core = dst // nodes_pc
    loc = dst - core * nodes_pc
    w = loc // win
    n_in_w = loc - w * win
    hi = (src >= half).astype(np.int64)

    key = (core * wpc + w) * 2 + hi
    order = np.argsort(key, kind="stable")
    counts = np.bincount(key, minlength=nc_cores * wpc * 2).reshape(nc_cores, wpc, 2)

    g_lo = _cdiv_arr(counts[:, :, 0].max(axis=0), 128)
    g_hi = _cdiv_arr(counts[:, :, 1].max(axis=0), 128)
    empty = (g_lo + g_hi) == 0
    g_lo[empty] = 1                               # every window has >=1 group
    s_w = g_lo + g_hi
    base_g = np.concatenate([[0], np.cumsum(s_w)])
    totg = int(base_g[-1])
    totslots = totg * 128
    smax = int(s_w.max())

    # weight tables (f-half negated so psum = [-zf | zs])
    wsrc = np.concatenate([-Wf[0:F], Ws[0:F]], axis=1).astype(F16)          # [64,128]
    wdst = np.concatenate([-Wf[F:2 * F], Ws[F:2 * F]], axis=1)
    bcat = np.concatenate([-bf, bs])[None, :]
    wq = np.concatenate([wdst, bcat], axis=0).astype(F16)                   # [65,128]
    wd = np.concatenate([-Wf[2 * F:], Ws[2 * F:]], axis=1).astype(F16)      # [10,128]
    ident = np.eye(128, dtype=F16)
    iota = np.tile(np.arange(128, dtype=F16)[None, :], (128, smax))         # [128,smax*128]

    # feature tables padded to 128 cols for xbar DMA-transpose
    feat16 = np.zeros((n_tab, 128), F16)
    feat16[:n_nodes, :F] = feature.astype(F16)

    per_core = []
    core_s, w_s, hi_s = core[order], w[order], hi[order]
    src_s, niw_s = src[order], n_in_w[order]
    dist_s = dist[order]

    for c in range(nc_cores):
        sidx = np.zeros(totslots, np.int16)
        Lhost = np.zeros((128, totslots), F16)
        dstloc = np.full(totslots, -5.0, F16)   # flat (g*128+p)

        sel = core_s == c
        cw, chi, csrc, cniw = w_s[sel], hi_s[sel], src_s[sel], niw_s[sel]
        cdist = dist_s[sel]
        ckey = cw * 2 + chi
        cnt = counts[c].reshape(-1)
        off = np.concatenate([[0], np.cumsum(cnt)])
        pos = np.arange(len(ckey)) - off[ckey]
        gcol = base_g[cw] + np.where(chi == 1, g_lo[cw], 0) + pos // 128
        p = pos % 128
        slot = gcol * 128 + p

        sidx[slot] = (csrc - np.where(chi == 1, half, 0)).astype(np.int16)
        dstloc[slot] = cniw.astype(F16)
        Lhost[cniw, slot] = 1.0
        Lhost[win + np.arange(D)[:, None], slot[None, :]] = cdist.T.astype(F16)

        frange = np.zeros((nodes_pc, F), np.float32)
        lo_n, hi_n = c * nodes_pc, min((c + 1) * nodes_pc, n_nodes)
        if hi_n > lo_n:
            frange[: hi_n - lo_n] = feature[lo_n:hi_n]
        featloc = np.zeros((qrows, 128), F16)
        featloc[:, F] = 1.0                      # bias-ones row after transpose
        hi_q = min(lo_n + qrows, n_nodes)
        if hi_q > lo_n:
            featloc[: hi_q - lo_n, :F] = feature[lo_n:hi_q].astype(F16)

        per_core.append({
            "feat16": feat16,
            "featloc": featloc,
            "frange": frange,
            "sidx": np.tile(sidx.reshape(totslots // 16, 16).T, (8, 1)).copy(),
            "Lmat": Lhost,
            "dstloc": dstloc.reshape(totg, 128).T.copy(),
            "wsrc": wsrc, "wq": wq, "wd": wd,
            "ident": ident, "iota": iota,
        })

    meta = {
        "g_lo": g_lo.tolist(), "g_hi": g_hi.tolist(),
        "base_g": base_g.tolist(), "totg": totg, "smax": smax,
        "n_tab": n_tab, "qrows": qrows, "win": win, "wpc": wpc,
        "nodes_pc": nodes_pc, "half": half,
    }
    return per_core, meta


# ============================================================== program builder
def build_program(meta, nc_cores=NC, phases=('pq', 'edge'), repeat=1):
    import concourse.tile as tile
    import concourse.mybir as mybir
    from concourse import bacc
    from concourse.bass import ts

    dt = mybir.dt
    AF = mybir.ActivationFunctionType
    ALU = mybir.AluOpType

    g_lo, g_hi = meta["g_lo"], meta["g_hi"]
    base_g = meta["base_g"]
    totg, smax = meta["totg"], meta["smax"]
    n_tab, qrows = meta["n_tab"], meta["qrows"]
    win, wpc, nodes_pc, half = meta["win"], meta["wpc"], meta["nodes_pc"], meta["half"]
    totslots = totg * 128

    import concourse.mybir as _mb
    import bass_rust as _br

    class _Bacc(bacc.Bacc):
        # Pin every activation to the one set holding Copy+Exp+Ln so the
        # table pass emits a single load instead of thrashing (2.7us/load).
        def insert_act_table_loads(self):
            from concourse.hw_specs import get_activation_tables
            has_act = any(isinstance(i, _mb.InstActivation)
                          for b in self.main_func.blocks for i in b.instructions)
            if not has_act:
                return
            tables = list(get_activation_tables(self.m.arch).items())
            keep = "natural_log_exp_and_others"
            filtered = [(n, (f if n == keep else set())) for n, f in tables]
            _br.insert_act_table_loads(self, filtered)

    nc = _Bacc("TRN2", target_bir_lowering=False, debug=False,
               num_devices=nc_cores, num_swdge_queues=4)

    f16, f32, i16 = dt.float16, dt.float32, dt.int16

    feat16 = nc.dram_tensor("feat16", [n_tab, 128], f16, kind="ExternalInput").ap()
    featloc = nc.dram_tensor("featloc", [qrows, 128], f16, kind="ExternalInput").ap()
    frange = nc.dram_tensor("frange", [nodes_pc, F], f32, kind="ExternalInput").ap()
    sidx_d = nc.dram_tensor("sidx", [128, totslots // 16], i16, kind="ExternalInput").ap()
    L_d = nc.dram_tensor("Lmat", [128, totslots], f16, kind="ExternalInput").ap()
    dstloc_d = nc.dram_tensor("dstloc", [128, totg], f16, kind="ExternalInput").ap()
    wsrc_d = nc.dram_tensor("wsrc", [F, 128], f16, kind="ExternalInput").ap()
    wq_d = nc.dram_tensor("wq", [F + 1, 128], f16, kind="ExternalInput").ap()
    wd_d = nc.dram_tensor("wd", [D, 128], f16, kind="ExternalInput").ap()
    ident_d = nc.dram_tensor("ident", [128, 128], f16, kind="ExternalInput").ap()
    iota_d = nc.dram_tensor("iota", [128, smax * 128], f16, kind="ExternalInput").ap()
    out_d = nc.dram_tensor("out", [nodes_pc, F], f32, kind="ExternalOutput").ap()

    P_d = nc.dram_tensor("P_tab", [n_tab, 128], f16).ap()
    Q_d = nc.dram_tensor("Q_tab", [qrows, 128], f16).ap()

    with tile.TileContext(nc) as tc:
        from contextlib import ExitStack
        with ExitStack() as ctx:
            if repeat > 1:
                ctx.enter_context(tc.For_i(0, repeat, 1))
            consts = ctx.enter_context(tc.tile_pool(name="consts", bufs=1))
            wsrc_t = consts.tile([F, 128], f16)
            nc.sync.dma_start(wsrc_t[:], wsrc_d[:])
            wq_t = consts.tile([F + 1, 128], f16)
            nc.sync.dma_start(wq_t[:], wq_d[:])
            ident_t = consts.tile([128, 128], f16)
            nc.sync.dma_start(ident_t[:], ident_d[:])
            iota_t = consts.tile([128, smax * 128], f16)
            nc.sync.dma_start(iota_t[:], iota_d[:])

            # ---------------- phase P/Q: node tables -------------------------
            with tc.tile_pool(name="pphase", bufs=3) as pp, \
                 tc.tile_pool(name="ppsum", bufs=2, space="PSUM") as pps:
                for a in range(0, n_tab, SLAB):
                    featT = pp.tile([128, SLAB], f16, tag="featT")
                    nc.sync.dma_start(featT[:], feat16[a:a + SLAB, :], transpose=True)
                    psum = pps.tile([128, SLAB], f32, tag="pp")
                    for j in range(SLAB // 128):
                        nc.tensor.matmul(psum[:, ts(j, 128)],
                                         lhsT=featT[0:F, ts(j, 128)],
                                         rhs=wsrc_t[:], start=True, stop=True)
                    pout = pp.tile([128, SLAB], f16, tag="pout")
                    nc.scalar.activation(pout[:], psum[:], AF.Copy)
                    nc.sync.dma_start(
                        P_d[a:a + SLAB, :].rearrange("(j p) f -> p j f", p=128),
                        pout[:].rearrange("p (j f) -> p j f", f=128))

                for a in range(0, qrows, SLAB):
                    sz = min(SLAB, qrows - a)
                    featq = pp.tile([128, SLAB], f16, tag="featT")
                    nc.sync.dma_start(featq[:, 0:sz], featloc[a:a + sz, :],
                                      transpose=True)
                    psum = pps.tile([128, SLAB], f32, tag="pp")
                    for j in range(sz // 128):
                        nc.tensor.matmul(psum[:, ts(j, 128)],
                                         lhsT=featq[0:F + 1, ts(j, 128)],
                                         rhs=wq_t[:], start=True, stop=True)
                    qout = pp.tile([128, SLAB], f16, tag="pout")
                    nc.scalar.activation(qout[:, 0:sz], psum[:, 0:sz], AF.Copy)
                    nc.sync.dma_start(
                        Q_d[a:a + sz, :].rearrange("(j p) f -> p j f", p=128),
                        qout[:, 0:sz].rearrange("p (j f) -> p j f", f=128))

            # ---------------- phase E: edges ---------------------------------
            if 'edge' not in phases:
                with tc.tile_pool(name='dummy', bufs=1) as dp:
                    zt = dp.tile([128, F], dt.float32)
                    nc.gpsimd.memset(zt[:], 0.0)
                    for w in range(wpc):
                        nc.sync.dma_start(out_d[w * win:(w + 1) * win, :], zt[0:win, :])
                return_early = True
            else:
                return_early = False
            if not return_early:
             with tc.tile_pool(name="ewin", bufs=3) as ew, \
                 tc.tile_pool(name="ebatch", bufs=4) as eb, \
                 tc.tile_pool(name="zpsum", bufs=2, space="PSUM") as zps, \
                 tc.tile_pool(name="mpsum", bufs=3, space="PSUM") as mps, \
                 tc.tile_pool(name="eout", bufs=3) as eo:
                _gq = [0]
                for w in range(wpc):
                    glo, ghi = g_lo[w], g_hi[w]
                    S = glo + ghi
                    g0 = base_g[w]

                    gsrc = ew.tile([128, S * 128], f16, tag="gsrc")
                    sidx_t = ew.tile([128, S * 8], i16, tag="sidx")
                    nc.sync.dma_start(sidx_t[:], sidx_d[:, g0 * 8:(g0 + S) * 8])
                    if 'nogather' in phases:
                        nc.gpsimd.memset(gsrc[:], 0.25)
                    if 'nogather' not in phases:
                        GMAX = 8   # ring holds 1024 descriptors per call
                        spans = [(g, min(g + GMAX, glo), P_d[0:min(half, n_tab), :])
                                 for g in range(0, glo, GMAX)]
                        spans += [(g, min(g + GMAX, S), P_d[half:n_tab, :])
                                  for g in range(glo, S, GMAX)]
                        for ga, gb, src_ap in spans:
                            ng = gb - ga
                            nc.gpsimd.dma_gather(
                                out_ap=gsrc[:, ga * 128:gb * 128]
                                .rearrange("p (c e) -> p c e", e=128),
                                in_ap=src_ap,
                                idxs_ap=sidx_t[:, ga * 8:gb * 8],
                                num_idxs=ng * 128, num_idxs_reg=ng * 128,
                                elem_size=128, queue_num=_gq[0] % 4)
                            _gq[0] += 1

                    L_t = ew.tile([128, S * 128], f16, tag="L")
                    nc.sync.dma_start(L_t[:], L_d[:, g0 * 128:(g0 + S) * 128])
                    dl_t = ew.tile([128, S], f16, tag="dl")
                    nc.sync.dma_start(dl_t[:], dstloc_d[:, g0:g0 + S])
                    R_t = ew.tile([128, 128], f16, tag="R")
                    nc.sync.dma_start(R_t[0:win, :],
                                      Q_d[w * win:(w + 1) * win, :])
                    nc.sync.dma_start(R_t[win:128, :], wd_d[:])

                    oh_t = ew.tile([128, S * 128], f16, tag="oh")
                    nc.vector.tensor_tensor(
                        out=oh_t[:].rearrange("p (s e) -> p s e", e=128),
                        in0=dl_t[:, :, None].to_broadcast([128, S, 128]),
                        in1=iota_t[:, 0:S * 128].rearrange("p (s e) -> p s e", e=128),
                        op=ALU.is_equal)

                    msum = mps.tile([win, F], f32, tag="msum")

                    for b0 in range(0, S, BATCH):
                        nb = min(BATCH, S - b0)
                        zp = zps.tile([128, BATCH * 128], f32, tag="zp")
                        for j in range(nb):
                            g = b0 + j
                            nc.tensor.matmul(zp[:, ts(j, 128)],
                                             lhsT=L_t[:, ts(g, 128)], rhs=R_t[:],
                                             start=True, stop=False)
                            nc.tensor.matmul(zp[:, ts(j, 128)], lhsT=ident_t[:],
                                             rhs=gsrc[:, ts(g, 128)],
                                             start=False, stop=True)
                        ez = eb.tile([128, BATCH * 128], f16, tag="ez")
                        nc.scalar.activation(ez[:, 0:nb * 128], zp[:, 0:nb * 128],
                                             AF.Exp)
                        # sp = ln(1 + e^{zs});  d = 1 + e^{-zf};  gated = sp/d
                        ezv = ez[:, 0:nb * 128].rearrange("p (j e) -> p j e", e=128)
                        sp_t = eb.tile([128, BATCH * F], f16, tag="sp")
                        spv = sp_t[:, 0:nb * F].rearrange("p (j e) -> p j e", e=F)
                        nc.scalar.activation(spv, ezv[:, :, F:128], AF.Ln, bias=1.0)
                        d_t = eb.tile([128, BATCH * F], f32, tag="d")
                        dv = d_t[:, 0:nb * F].rearrange("p (j e) -> p j e", e=F)
                        nc.vector.tensor_scalar(out=dv, in0=ezv[:, :, 0:F],
                                                scalar1=1.0, scalar2=None,
                                                op0=ALU.add)
                        r_t = eb.tile([128, BATCH * F], f32, tag="r")
                        nc.vector.reciprocal_approx_fast(r_t[:, 0:nb * F],
                                                         d_t[:, 0:nb * F])
                        gat = eb.tile([128, BATCH * F], f16, tag="gat")
                        nc.vector.tensor_tensor(out=gat[:, 0:nb * F],
                                                in0=sp_t[:, 0:nb * F],
                                                in1=r_t[:, 0:nb * F], op=ALU.mult)
                        gatv = gat[:, 0:nb * F].rearrange("p (j e) -> p j e", e=F)
                        for j in range(nb):
                            g = b0 + j
                            nc.tensor.matmul(
                                msum[:], lhsT=oh_t[:, g * 128:g * 128 + win],
                                rhs=gatv[:, j, :],
                                start=(g == 0), stop=(g == S - 1))

                    fr_t = eo.tile([win, F], f32, tag="fr")
                    nc.sync.dma_start(fr_t[:], frange[w * win:(w + 1) * win, :])
                    o_t = eo.tile([win, F], f32, tag="o")
                    nc.vector.tensor_tensor(out=o_t[:], in0=msum[:], in1=fr_t[:],
                                            op=ALU.add)
                    nc.sync.dma_start(out_d[w * win:(w + 1) * win, :], o_t[:])

    nc.compile()
    return nc


# ===================================================================== kernel()
_CACHE = {}


def kernel(**inputs):
    per_core, meta = preprocess(
        inputs["feature"], inputs["dist"], inputs["src"], inputs["dst"],
        inputs["Wf"], inputs["bf"], inputs["Ws"], inputs["bs"])

    key = (meta["totg"], tuple(meta["g_lo"]), tuple(meta["g_hi"]))
    if key not in _CACHE:
        _CACHE.clear()
        _CACHE[key] = build_program(meta)
    nc = _CACHE[key]

    from concourse.bass_utils import run_bass_kernel_spmd
    res = run_bass_kernel_spmd(nc, per_core, list(range(NC)))

    outs = [res.results[c]["out"] for c in range(NC)]
    full = np.concatenate(outs, axis=0)[:N_NODES]
    return np.asarray(full, np.float32)


# revision 16
# speedup vs baseline: 1.5434x; 1.4884x over previous
"""GNN message-passing (CG-CNN layer) Trainium2 kernel.

out = feature + segment_sum(sigmoid(msg@Wf+bf) * softplus(msg@Ws+bs), dst)
where msg = [feature[src], feature[dst], dist].

Strategy (8 NeuronCores, SPMD, no collectives):
- Shard nodes by dst-range: core c owns nodes [c*6254, (c+1)*6254). Each core
  receives exactly the edges whose dst is in its range, grouped into windows
  of 118 dst-nodes.
- Per-node precompute (on device): P[n] = feature[n] @ [-Wf_src | Ws_src] and
  Q[n] = feature[n] @ [-Wf_dst | Ws_dst] + [-bf | bs]  (fp16 tables in DRAM).
  The f-half signs are flipped so the psum holds [-zf | zs] and a single
  full-width exp() serves both gates.
- Per edge group of 128: z = MM_A(L_g, R_w) + MM_B(I, P[src]-gather) where
  L_g = [onehot(dst_local) ; dist^T] (host-built fp16) and R_w = [Q_win ; Wd].
- gated = ln(1+e^{zs}) * 1/(1+e^{-zf}) using the exp/ln ACT table set only.
- scatter: m_sum_win += onehot^T @ gated via PE matmul accumulation in PSUM
  (race-free), then out_win = m_sum + feature[win] written densely.
"""

import sys

sys.path.insert(0, "/opt/trn_rl_repo")

import numpy as np

F16 = np.float16

# ---------------------------------------------------------------- problem dims
N_NODES = 50000
N_EDGES = 800000
F = 64
D = 10
NC = 8

WIN = 118          # dst-nodes per window (K budget: 118 + 10 = 128)
WPC = 53           # windows per core
HALF = 32768       # int16 gather index budget
BATCH = 8          # edge groups per psum batch
SLAB = 1024        # node-table build slab


def _cdiv(a, b):
    return (a + b - 1) // b


def _cdiv_arr(a, b):
    return -(-a // b)


# ============================================================ host preprocessing
def preprocess(feature, dist, src, dst, Wf, bf, Ws, bs,
               n_nodes=N_NODES, nc_cores=NC, win=WIN, wpc=WPC, half=HALF):
    """Pure layout/indexing prep on host. Returns (per_core_inputs, meta)."""
    nodes_pc = win * wpc
    assert nc_cores * nodes_pc >= n_nodes

    feature = np.asarray(feature, np.float32)
    dist = np.asarray(dist, np.float32)
    src = np.asarray(src).astype(np.int64)
    dst = np.asarray(dst).astype(np.int64)
    Wf = np.asarray(Wf, np.float32)
    bf = np.asarray(bf, np.float32)
    Ws = np.asarray(Ws, np.float32)
    bs = np.asarray(bs, np.float32)

    n_tab = _cdiv(n_nodes, SLAB) * SLAB          # P table rows (slab-aligned)
    qrows = _cdiv(nodes_pc, 128) * 128           # Q table rows per core

    core = dst // nodes_pc
    loc = dst - core * nodes_pc
    w = loc // win
    n_in_w = loc - w * win
    hi = (src >= half).astype(np.int64)

    key = (core * wpc + w) * 2 + hi
    order = np.argsort(key, kind="stable")
    counts = np.bincount(key, minlength=nc_cores * wpc * 2).reshape(nc_cores, wpc, 2)

    g_lo = _cdiv_arr(counts[:, :, 0].max(axis=0), 128)
    g_hi = _cdiv_arr(counts[:, :, 1].max(axis=0), 128)
    empty = (g_lo + g_hi) == 0
    g_lo[empty] = 1                               # every window has >=1 group
    s_w = g_lo + g_hi
    base_g = np.concatenate([[0], np.cumsum(s_w)])
    totg = int(base_g[-1])
    totslots = totg * 128
    smax = int(s_w.max())

    # weight tables (f-half negated so psum = [-zf | zs])
    wsrc = np.concatenate([-Wf[0:F], Ws[0:F]], axis=1).astype(F16)          # [64,128]
    wdst = np.concatenate([-Wf[F:2 * F], Ws[F:2 * F]], axis=1)
    bcat = np.concatenate([-bf, bs])[None, :]
    wq = np.concatenate([wdst, bcat], axis=0).astype(F16)                   # [65,128]
    wd = np.concatenate([-Wf[2 * F:], Ws[2 * F:]], axis=1).astype(F16)      # [10,128]
    ident = np.eye(128, dtype=F16)
    iota = np.tile(np.arange(128, dtype=F16)[None, :], (128, smax))         # [128,smax*128]

    # feature tables padded to 128 cols for xbar DMA-transpose
    feat16 = np.zeros((n_tab, 128), F16)
    feat16[:n_nodes, :F] = feature.astype(F16)

    per_core = []
    core_s, w_s, hi_s = core[order], w[order], hi[order]
    src_s, niw_s = src[order], n_in_w[order]
    dist_s = dist[order]

    for c in range(nc_cores):
        sidx = np.zeros(totslots, np.int16)
        Lhost = np.zeros((128, totslots), F16)
        dstloc = np.full(totslots, -5.0, F16)   # flat (g*128+p)

        sel = core_s == c
        cw, chi, csrc, cniw = w_s[sel], hi_s[sel], src_s[sel], niw_s[sel]
        cdist = dist_s[sel]
        ckey = cw * 2 + chi
        cnt = counts[c].reshape(-1)
        off = np.concatenate([[0], np.cumsum(cnt)])
        pos = np.arange(len(ckey)) - off[ckey]
        gcol = base_g[cw] + np.where(chi == 1, g_lo[cw], 0) + pos // 128
        p = pos % 128
        slot = gcol * 128 + p

        sidx[slot] = (csrc - np.where(chi == 1, half, 0)).astype(np.int16)
        dstloc[slot] = cniw.astype(F16)
        Lhost[cniw, slot] = 1.0
        Lhost[win + np.arange(D)[:, None], slot[None, :]] = cdist.T.astype(F16)

        frange = np.zeros((nodes_pc, F), np.float32)
        lo_n, hi_n = c * nodes_pc, min((c + 1) * nodes_pc, n_nodes)
        if hi_n > lo_n:
            frange[: hi_n - lo_n] = feature[lo_n:hi_n]
        featloc = np.zeros((qrows, 128), F16)
        featloc[:, F] = 1.0                      # bias-ones row after transpose
        hi_q = min(lo_n + qrows, n_nodes)
        if hi_q > lo_n:
            featloc[: hi_q - lo_n, :F] = feature[lo_n:hi_q].astype(F16)

        per_core.append({
            "feat16": feat16,
            "featloc": featloc,
            "frange": frange,
            "sidx": np.tile(sidx.reshape(totslots // 16, 16).T, (8, 1)).copy(),
            "Lmat": Lhost,
            "dstloc": dstloc.reshape(totg, 128).T.copy(),
            "wsrc": wsrc, "wq": wq, "wd": wd,
            "ident": ident, "iota": iota,
        })

    meta = {
        "g_lo": g_lo.tolist(), "g_hi": g_hi.tolist(),
        "base_g": base_g.tolist(), "totg": totg, "smax": smax,
        "n_tab": n_tab, "qrows": qrows, "win": win, "wpc": wpc,
        "nodes_pc": nodes_pc, "half": half,
    }
    return per_core, meta


# ============================================================== program builder
def build_program(meta, nc_cores=NC, phases=('pq', 'edge'), repeat=1):
    import concourse.tile as tile
    import concourse.mybir as mybir
    from concourse import bacc
    from concourse.bass import ts

    dt = mybir.dt
    AF = mybir.ActivationFunctionType
    ALU = mybir.AluOpType

    g_lo, g_hi = meta["g_lo"], meta["g_hi"]
    base_g = meta["base_g"]
    totg, smax = meta["totg"], meta["smax"]
    n_tab, qrows = meta["n_tab"], meta["qrows"]
    win, wpc, nodes_pc, half = meta["win"], meta["wpc"], meta["nodes_pc"], meta["half"]
    totslots = totg * 128

    import concourse.mybir as _mb
    import bass_rust as _br

    class _Bacc(bacc.Bacc):
        # Pin every activation to the one set holding Copy+Exp+Ln so the
        # table pass emits a single load instead of thrashing (2.7us/load).
        def insert_act_table_loads(self):
            from concourse.hw_specs import get_activation_tables
            has_act = any(isinstance(i, _mb.InstActivation)
                          for b in self.main_func.blocks for i in b.instructions)
            if not has_act:
                return
            tables = list(get_activation_tables(self.m.arch).items())
            keep = "natural_log_exp_and_others"
            filtered = [(n, (f if n == keep else set())) for n, f in tables]
            _br.insert_act_table_loads(self, filtered)

    nc = _Bacc("TRN2", target_bir_lowering=False, debug=False,
               num_devices=nc_cores, num_swdge_queues=4)

    f16, f32, i16 = dt.float16, dt.float32, dt.int16

    feat16 = nc.dram_tensor("feat16", [n_tab, 128], f16, kind="ExternalInput").ap()
    featloc = nc.dram_tensor("featloc", [qrows, 128], f16, kind="ExternalInput").ap()
    frange = nc.dram_tensor("frange", [nodes_pc, F], f32, kind="ExternalInput").ap()
    sidx_d = nc.dram_tensor("sidx", [128, totslots // 16], i16, kind="ExternalInput").ap()
    L_d = nc.dram_tensor("Lmat", [128, totslots], f16, kind="ExternalInput").ap()
    dstloc_d = nc.dram_tensor("dstloc", [128, totg], f16, kind="ExternalInput").ap()
    wsrc_d = nc.dram_tensor("wsrc", [F, 128], f16, kind="ExternalInput").ap()
    wq_d = nc.dram_tensor("wq", [F + 1, 128], f16, kind="ExternalInput").ap()
    wd_d = nc.dram_tensor("wd", [D, 128], f16, kind="ExternalInput").ap()
    ident_d = nc.dram_tensor("ident", [128, 128], f16, kind="ExternalInput").ap()
    iota_d = nc.dram_tensor("iota", [128, smax * 128], f16, kind="ExternalInput").ap()
    out_d = nc.dram_tensor("out", [nodes_pc, F], f32, kind="ExternalOutput").ap()

    P_d = nc.dram_tensor("P_tab", [n_tab, 128], f16).ap()
    Q_d = nc.dram_tensor("Q_tab", [qrows, 128], f16).ap()

    with tile.TileContext(nc) as tc:
        from contextlib import ExitStack
        with ExitStack() as ctx:
            if repeat > 1:
                ctx.enter_context(tc.For_i(0, repeat, 1))
            consts = ctx.enter_context(tc.tile_pool(name="consts", bufs=1))
            wsrc_t = consts.tile([F, 128], f16)
            nc.sync.dma_start(wsrc_t[:], wsrc_d[:])
            wq_t = consts.tile([F + 1, 128], f16)
            nc.sync.dma_start(wq_t[:], wq_d[:])
            ident_t = consts.tile([128, 128], f16)
            nc.sync.dma_start(ident_t[:], ident_d[:])
            iota_t = consts.tile([128, smax * 128], f16)
            nc.sync.dma_start(iota_t[:], iota_d[:])

            # ---------------- phase P/Q: node tables -------------------------
            with tc.tile_pool(name="pphase", bufs=3) as pp, \
                 tc.tile_pool(name="ppsum", bufs=2, space="PSUM") as pps:
                for a in range(0, n_tab, SLAB):
                    featT = pp.tile([128, SLAB], f16, tag="featT")
                    nc.sync.dma_start(featT[:], feat16[a:a + SLAB, :], transpose=True)
                    psum = pps.tile([128, SLAB], f32, tag="pp")
                    for j in range(SLAB // 128):
                        nc.tensor.matmul(psum[:, ts(j, 128)],
                                         lhsT=featT[0:F, ts(j, 128)],
                                         rhs=wsrc_t[:], start=True, stop=True)
                    pout = pp.tile([128, SLAB], f16, tag="pout")
                    nc.scalar.activation(pout[:], psum[:], AF.Copy)
                    nc.sync.dma_start(
                        P_d[a:a + SLAB, :].rearrange("(j p) f -> p j f", p=128),
                        pout[:].rearrange("p (j f) -> p j f", f=128))

                for a in range(0, qrows, SLAB):
                    sz = min(SLAB, qrows - a)
                    featq = pp.tile([128, SLAB], f16, tag="featT")
                    nc.sync.dma_start(featq[:, 0:sz], featloc[a:a + sz, :],
                                      transpose=True)
                    psum = pps.tile([128, SLAB], f32, tag="pp")
                    for j in range(sz // 128):
                        nc.tensor.matmul(psum[:, ts(j, 128)],
                                         lhsT=featq[0:F + 1, ts(j, 128)],
                                         rhs=wq_t[:], start=True, stop=True)
                    qout = pp.tile([128, SLAB], f16, tag="pout")
                    nc.scalar.activation(qout[:, 0:sz], psum[:, 0:sz], AF.Copy)
                    nc.sync.dma_start(
                        Q_d[a:a + sz, :].rearrange("(j p) f -> p j f", p=128),
                        qout[:, 0:sz].rearrange("p (j f) -> p j f", f=128))

            # ---------------- phase E: edges ---------------------------------
            if 'edge' not in phases:
                with tc.tile_pool(name='dummy', bufs=1) as dp:
                    zt = dp.tile([128, F], dt.float32)
                    nc.gpsimd.memset(zt[:], 0.0)
                    for w in range(wpc):
                        nc.sync.dma_start(out_d[w * win:(w + 1) * win, :], zt[0:win, :])
                return_early = True
            else:
                return_early = False
            if not return_early:
             with tc.tile_pool(name="ewin", bufs=3) as ew, \
                 tc.tile_pool(name="ebatch", bufs=4) as eb, \
                 tc.tile_pool(name="zpsum", bufs=2, space="PSUM") as zps, \
                 tc.tile_pool(name="mpsum", bufs=3, space="PSUM") as mps, \
                 tc.tile_pool(name="eout", bufs=3) as eo:
                _gq = [0]
                for w in range(wpc):
                    glo, ghi = g_lo[w], g_hi[w]
                    S = glo + ghi
                    g0 = base_g[w]

                    gsrc = ew.tile([128, S * 128], f16, tag="gsrc")
                    sidx_t = ew.tile([128, S * 8], i16, tag="sidx")
                    nc.sync.dma_start(sidx_t[:], sidx_d[:, g0 * 8:(g0 + S) * 8])
                    if 'nogather' in phases:
                        nc.gpsimd.memset(gsrc[:], 0.25)
                    if 'nogather' not in phases:
                        GMAX = 8   # ring holds 1024 descriptors per call
                        spans = [(g, min(g + GMAX, glo), P_d[0:min(half, n_tab), :])
                                 for g in range(0, glo, GMAX)]
                        spans += [(g, min(g + GMAX, S), P_d[half:n_tab, :])
                                  for g in range(glo, S, GMAX)]
                        for ga, gb, src_ap in spans:
                            ng = gb - ga
                            nc.gpsimd.dma_gather(
                                out_ap=gsrc[:, ga * 128:gb * 128]
                                .rearrange("p (c e) -> p c e", e=128),
                                in_ap=src_ap,
                                idxs_ap=sidx_t[:, ga * 8:gb * 8],
                                num_idxs=ng * 128, num_idxs_reg=ng * 128,
                                elem_size=128, queue_num=_gq[0] % 4)
                            _gq[0] += 1

                    L_t = ew.tile([128, S * 128], f16, tag="L")
                    nc.sync.dma_start(L_t[:], L_d[:, g0 * 128:(g0 + S) * 128])
                    dl_t = ew.tile([128, S], f16, tag="dl")
                    nc.sync.dma_start(dl_t[:], dstloc_d[:, g0:g0 + S])
                    R_t = ew.tile([128, 128], f16, tag="R")
                    nc.sync.dma_start(R_t[0:win, :],
                                      Q_d[w * win:(w + 1) * win, :])
                    nc.sync.dma_start(R_t[win:128, :], wd_d[:])

                    oh_t = ew.tile([128, S * 128], f16, tag="oh")
                    nc.vector.tensor_tensor(
                        out=oh_t[:].rearrange("p (s e) -> p s e", e=128),
                        in0=dl_t[:, :, None].to_broadcast([128, S, 128]),
                        in1=iota_t[:, 0:S * 128].rearrange("p (s e) -> p s e", e=128),
                        op=ALU.is_equal)

                    msum = mps.tile([win, F], f32, tag="msum")

                    if 'nocompute' in phases:
                        fr_t = eo.tile([win, F], f32, tag="fr")
                        nc.sync.dma_start(fr_t[:], frange[w * win:(w + 1) * win, :])
                        o_t = eo.tile([win, F], f32, tag="o")
                        nc.vector.tensor_tensor(out=o_t[:], in0=fr_t[:],
                                                in1=fr_t[:], op=ALU.add)
                        nc.sync.dma_start(out_d[w * win:(w + 1) * win, :], o_t[:])
                        continue

                    for b0 in range(0, S, BATCH):
                        nb = min(BATCH, S - b0)
                        zp = zps.tile([128, BATCH * 128], f32, tag="zp")
                        for j in range(nb):
                            g = b0 + j
                            nc.tensor.matmul(zp[:, ts(j, 128)],
                                             lhsT=L_t[:, ts(g, 128)], rhs=R_t[:],
                                             start=True, stop=False)
                            nc.tensor.matmul(zp[:, ts(j, 128)], lhsT=ident_t[:],
                                             rhs=gsrc[:, ts(g, 128)],
                                             start=False, stop=True)
                        if 'noact' in phases:
                            gatv = gsrc[:, 0:nb * F].rearrange(
                                "p (j e) -> p j e", e=F)
                            for j in range(nb):
                                g = b0 + j
                                nc.tensor.matmul(
                                    msum[:], lhsT=oh_t[:, g * 128:g * 128 + win],
                                    rhs=gatv[:, j, :],
                                    start=(g == 0), stop=(g == S - 1))
                            continue
                        ez = eb.tile([128, BATCH * 128], f16, tag="ez")
                        nc.scalar.activation(ez[:, 0:nb * 128], zp[:, 0:nb * 128],
                                             AF.Exp)
                        # sp = ln(1 + e^{zs});  d = 1 + e^{-zf};  gated = sp/d
                        ezv = ez[:, 0:nb * 128].rearrange("p (j e) -> p j e", e=128)
                        sp_t = eb.tile([128, BATCH * F], f16, tag="sp")
                        spv = sp_t[:, 0:nb * F].rearrange("p (j e) -> p j e", e=F)
                        nc.scalar.activation(spv, ezv[:, :, F:128], AF.Ln, bias=1.0)
                        d_t = eb.tile([128, BATCH * F], f32, tag="d")
                        dv = d_t[:, 0:nb * F].rearrange("p (j e) -> p j e", e=F)
                        nc.vector.tensor_scalar(out=dv, in0=ezv[:, :, 0:F],
                                                scalar1=1.0, scalar2=None,
                                                op0=ALU.add)
                        r_t = eb.tile([128, BATCH * F], f32, tag="r")
                        nc.vector.reciprocal_approx_fast(r_t[:, 0:nb * F],
                                                         d_t[:, 0:nb * F])
                        gat = eb.tile([128, BATCH * F], f16, tag="gat")
                        nc.vector.tensor_tensor(out=gat[:, 0:nb * F],
                                                in0=sp_t[:, 0:nb * F],
                                                in1=r_t[:, 0:nb * F], op=ALU.mult)
                        gatv = gat[:, 0:nb * F].rearrange("p (j e) -> p j e", e=F)
                        for j in range(nb):
                            g = b0 + j
                            nc.tensor.matmul(
                                msum[:], lhsT=oh_t[:, g * 128:g * 128 + win],
                                rhs=gatv[:, j, :],
                                start=(g == 0), stop=(g == S - 1))

                    fr_t = eo.tile([win, F], f32, tag="fr")
                    nc.sync.dma_start(fr_t[:], frange[w * win:(w + 1) * win, :])
                    o_t = eo.tile([win, F], f32, tag="o")
                    nc.vector.tensor_tensor(out=o_t[:], in0=msum[:], in1=fr_t[:],
                                            op=ALU.add)
                    nc.sync.dma_start(out_d[w * win:(w + 1) * win, :], o_t[:])

    nc.compile()
    return nc


# ===================================================================== kernel()
_CACHE = {}


def kernel(**inputs):
    per_core, meta = preprocess(
        inputs["feature"], inputs["dist"], inputs["src"], inputs["dst"],
        inputs["Wf"], inputs["bf"], inputs["Ws"], inputs["bs"])

    key = (meta["totg"], tuple(meta["g_lo"]), tuple(meta["g_hi"]))
    if key not in _CACHE:
        _CACHE.clear()
        _CACHE[key] = build_program(meta)
    nc = _CACHE[key]

    from concourse.bass_utils import run_bass_kernel_spmd
    res = run_bass_kernel_spmd(nc, per_core, list(range(NC)))

    outs = [res.results[c]["out"] for c in range(NC)]
    full = np.concatenate(outs, axis=0)[:N_NODES]
    return np.asarray(full, np.float32)
